# revision 3
# baseline (speedup 1.0000x reference)
"""Trainium2 Bass kernel for the Griffin-style gated linear recurrence.

Model (matching the jax reference, including its chunked-scan numerics):
    a = sigmoid(x @ Wa.T + decay_bias)
    i = sigmoid(x @ Wi.T)
    v = x @ Wv.T
    w = sqrt(max(1 - a*a, 1e-8)) * i * v
    chunked scan (chunk=64), algebraically equal to
    h[t] = a[t]*h[t-1] + g[t]*w[t],  g[t] = min(1, cd[t]*1e10),
    cd = within-chunk running product of a (reset every 64 steps).

Sharding: 4 batches x 2 channel-halves = 8 cores, no communication.

Per-core layout: channels on partitions, time on the free axis, blocks of
SB=512 timesteps.  The 576 projection columns (3 gates x 192 channels) are
packed into 5 stationary tiles per k-tile instead of 6:
    z0 = a[0:128]   z1 = i[0:128]   z2 = v[0:128]
    z3 = [a[128:192] ; i[128:192]]  (64+64 partitions, one sigmoid with a
                                     [bias_hi ; 0] per-partition bias)
    z4 = v[128:192]                  (64 partitions)
which cuts tensor-engine time by 1/6 (matmul cost is N-cycles per
instruction regardless of M).  Inputs stream in bf16 (PSUM accumulates
fp32); the decay path (a, a^2, cd, h) stays fp32 end-to-end because
sqrt(1-a^2) for slow channels is catastrophically cancellation-sensitive.
The iv path runs bf16 where both operands are 2-byte (2x DVE mode).

z3 holds a_hi/i_hi on different partitions, so one cross-partition
SBUF->SBUF DMA per block realigns i_hi with v_hi (engines are lane-locked;
only DMA can move data across partitions).

Engine balance per block (PE is the bottleneck at ~8.5us/block):
    PE  : 40 bf16 matmuls (5 tiles x 8 k-steps, N=512).
    Act : 3 sigmoids + 2 sqrt + ~2 act-table loads (sigmoid and sqrt never
          share a hardware table; the Tile scheduler is table-blind so the
          swap per block is unavoidable).
    DVE : u muls (PSUM reads), w/gw muls (bf16 2x), 16 cd chunk-scans,
          2 h scans (scan is DVE-only; ISA rejects it on Pool).
    Pool: a^2 squares and the g = min(cd*1e10, 1) clamps.
"""

import sys

if "/opt/trn_rl_repo" not in sys.path:
    sys.path.insert(0, "/opt/trn_rl_repo")

from contextlib import ExitStack

import numpy as np
import ml_dtypes

from concourse import bacc, bass, mybir, tile
from concourse.bass_utils import run_bass_kernel_spmd

B, S = 4, 4096
DM, DR = 1024, 384
DC = DR // 2          # channels per core
CH = 64               # scan chunk size
SB = 512              # sequence block per pipeline step
NB = S // SB
KT = DM // 128        # contraction tiles

F32 = mybir.dt.float32
BF16 = mybir.dt.bfloat16
AFT = mybir.ActivationFunctionType
OP = mybir.AluOpType

# column ranges of the 5 packed stationary tiles
TCOLS = ((0, 128), (128, 256), (256, 384), (384, 512), (512, 576))

_CACHED_NC = None


def _build_nc():
    nc = bacc.Bacc(trn_type="TRN2")

    xT = nc.dram_tensor("xt", [DM, S], BF16, kind="ExternalInput")
    wc = nc.dram_tensor("wcat", [DM, 576], BF16, kind="ExternalInput")
    bias0 = nc.dram_tensor("bias0", [128, 1], F32, kind="ExternalInput")
    bias3 = nc.dram_tensor("bias3", [128, 1], F32, kind="ExternalInput")
    out = nc.dram_tensor("out", [DC, S], F32, kind="ExternalOutput")

    with tile.TileContext(nc) as tc, ExitStack() as ctx:
        wp = ctx.enter_context(tc.tile_pool(name="wp", bufs=1))
        cp = ctx.enter_context(tc.tile_pool(name="cp", bufs=1))
        xp = ctx.enter_context(tc.tile_pool(name="xp", bufs=3))
        pp = ctx.enter_context(tc.tile_pool(name="pp", bufs=1, space="PSUM"))
        sp = ctx.enter_context(tc.tile_pool(name="sp", bufs=2))
        hp = ctx.enter_context(tc.tile_pool(name="hp", bufs=2))

        # --- constants -------------------------------------------------
        w_sb = wp.tile([128, KT, 576], BF16, tag="wc")
        nc.sync.dma_start(w_sb[:], wc.rearrange("(k p) c -> p k c", p=128))

        b0 = cp.tile([128, 1], F32, tag="b0")
        nc.sync.dma_start(b0[:], bias0[:, :])
        b3 = cp.tile([128, 1], F32, tag="b3")
        nc.sync.dma_start(b3[:], bias3[:, :])

        zeros = cp.tile([128, CH], F32, tag="zeros")
        nc.vector.memset(zeros[:], 0.0)

        prev_h = None
        for ib in range(NB):
            s0 = ib * SB

            x_sb = xp.tile([128, KT, SB], BF16, tag="x")
            nc.sync.dma_start(
                x_sb[:],
                xT.rearrange("(k p) s -> p k s", p=128)[:, :, s0:s0 + SB])

            z = []
            for t, (c0, c1) in enumerate(TCOLS):
                zt = pp.tile([c1 - c0, SB], F32, tag=f"z{t}")
                for k in range(KT):
                    nc.tensor.matmul(
                        zt[:],
                        w_sb[:, k, c0:c1],
                        x_sb[:, k, :],
                        start=(k == 0),
                        stop=(k == KT - 1),
                    )
                z.append(zt)

            # sigmoids (drain z0, z1, z3)
            a_lo = sp.tile([128, SB], F32, tag="a_lo")
            nc.scalar.activation(a_lo[:], z[0][:], AFT.Sigmoid, bias=b0[:])
            i_lo = sp.tile([128, SB], F32, tag="i_lo")
            nc.scalar.activation(i_lo[:], z[1][:], AFT.Sigmoid)
            s3 = sp.tile([128, SB], F32, tag="s3")     # [a_hi ; i_hi]
            nc.scalar.activation(s3[:], z[3][:], AFT.Sigmoid, bias=b3[:])
            a_hi = s3[0:64, :]

            # realign i_hi (partitions 64:128) with v_hi (partitions 0:64)
            ic = sp.tile([64, SB], F32, tag="ic")
            nc.sync.dma_start(ic[:], s3[64:128, :])

            # squares on Pool
            m_lo = sp.tile([128, SB], F32, tag="m_lo")
            nc.gpsimd.tensor_mul(m_lo[:], a_lo[:], a_lo[:])
            m_hi = sp.tile([64, SB], F32, tag="m_hi")
            nc.gpsimd.tensor_mul(m_hi[:], a_hi, a_hi)

            # r = sqrt(1 - a^2); 1 - a*a stays far above the reference's
            # 1e-8 floor for every reachable a, so the max() is a no-op.
            r_lo = sp.tile([128, SB], BF16, tag="r_lo")
            nc.scalar.activation(r_lo[:], m_lo[:], AFT.Sqrt, bias=1.0,
                                 scale=-1.0)
            r_hi = sp.tile([64, SB], BF16, tag="r_hi")
            nc.scalar.activation(r_hi[:], m_hi[:], AFT.Sqrt, bias=1.0,
                                 scale=-1.0)

            # u = i * v straight out of PSUM (drains z2, z4)
            u_lo = sp.tile([128, SB], BF16, tag="u_lo")
            nc.vector.tensor_mul(u_lo[:], i_lo[:], z[2][:])
            u_hi = sp.tile([64, SB], BF16, tag="u_hi")
            nc.vector.tensor_mul(u_hi[:], ic[:], z[4][:])

            # within-chunk running product of a: one scan per 64-chunk
            cd_lo = sp.tile([128, SB], F32, tag="cd_lo")
            cd_hi = sp.tile([64, SB], F32, tag="cd_hi")
            for c in range(SB // CH):
                cs = slice(c * CH, (c + 1) * CH)
                nc.vector.tensor_tensor_scan(
                    cd_lo[:, cs], a_lo[:, cs], zeros[0:128, :], 1.0,
                    op0=OP.mult, op1=OP.add)
            for c in range(SB // CH):
                cs = slice(c * CH, (c + 1) * CH)
                nc.vector.tensor_tensor_scan(
                    cd_hi[:, cs], s3[0:64, cs], zeros[0:64, :], 1.0,
                    op0=OP.mult, op1=OP.add)

            w_lo = sp.tile([128, SB], BF16, tag="w_lo")
            nc.vector.tensor_mul(w_lo[:], r_lo[:], u_lo[:])
            w_hi = sp.tile([64, SB], BF16, tag="w_hi")
            nc.vector.tensor_mul(w_hi[:], r_hi[:], u_hi[:])

            # g = min(cd * 1e10, 1) == cd / max(cd, 1e-10), on Pool
            g_lo = sp.tile([128, SB], BF16, tag="g_lo")
            nc.gpsimd.tensor_scalar(
                g_lo[:], cd_lo[:], 1e10, 1.0, op0=OP.mult, op1=OP.min)
            g_hi = sp.tile([64, SB], BF16, tag="g_hi")
            nc.gpsimd.tensor_scalar(
                g_hi[:], cd_hi[:], 1e10, 1.0, op0=OP.mult, op1=OP.min)

            gw_lo = sp.tile([128, SB], BF16, tag="gw_lo")
            nc.vector.tensor_mul(gw_lo[:], g_lo[:], w_lo[:])
            gw_hi = sp.tile([64, SB], BF16, tag="gw_hi")
            nc.vector.tensor_mul(gw_hi[:], g_hi[:], w_hi[:])

            h_lo = hp.tile([128, SB], F32, tag="h_lo")
            init_lo = 0.0 if prev_h is None else prev_h[0][:, SB - 1:SB]
            nc.vector.tensor_tensor_scan(
                h_lo[:], a_lo[:], gw_lo[:], init_lo, op0=OP.mult, op1=OP.add)
            h_hi = hp.tile([64, SB], F32, tag="h_hi")
            init_hi = 0.0 if prev_h is None else prev_h[1][:, SB - 1:SB]
            nc.vector.tensor_tensor_scan(
                h_hi[:], s3[0:64, :], gw_hi[:], init_hi,
                op0=OP.mult, op1=OP.add)

            nc.sync.dma_start(out[0:128, s0:s0 + SB], h_lo[:])
            nc.sync.dma_start(out[128:DC, s0:s0 + SB], h_hi[:])
            prev_h = (h_lo, h_hi)

    nc.finalize()
    return nc


def _make_in_maps(x, Wa, Wi, Wv, decay_bias):
    x = np.asarray(x, dtype=np.float32)
    Wa = np.asarray(Wa, dtype=np.float32)
    Wi = np.asarray(Wi, dtype=np.float32)
    Wv = np.asarray(Wv, dtype=np.float32)
    decay_bias = np.asarray(decay_bias, dtype=np.float32)

    in_maps = []
    for b in range(B):
        xTb = np.ascontiguousarray(x[b].T).astype(ml_dtypes.bfloat16)
        for j in range(2):
            c0 = j * DC
            wcat = np.concatenate(
                [
                    Wa[c0:c0 + 128].T,
                    Wi[c0:c0 + 128].T,
                    Wv[c0:c0 + 128].T,
                    Wa[c0 + 128:c0 + DC].T,
                    Wi[c0 + 128:c0 + DC].T,
                    Wv[c0 + 128:c0 + DC].T,
                ],
                axis=1,
            ).astype(ml_dtypes.bfloat16)
            b0 = np.ascontiguousarray(decay_bias[c0:c0 + 128, None])
            b3 = np.zeros((128, 1), dtype=np.float32)
            b3[0:64, 0] = decay_bias[c0 + 128:c0 + DC]
            in_maps.append({
                "xt": xTb,
                "wcat": np.ascontiguousarray(wcat),
                "bias0": b0,
                "bias3": b3,
            })
    return in_maps


def kernel(x, Wa, Wi, Wv, decay_bias):
    global _CACHED_NC
    if _CACHED_NC is None:
        _CACHED_NC = _build_nc()
    nc = _CACHED_NC

    in_maps = _make_in_maps(x, Wa, Wi, Wv, decay_bias)
    res = run_bass_kernel_spmd(nc, in_maps, core_ids=list(range(8)))

    out = np.empty((B, S, DR), dtype=np.float32)
    for b in range(B):
        for j in range(2):
            core = 2 * b + j
            out[b, :, j * DC:(j + 1) * DC] = res.results[core]["out"].T
    return out


# revision 9
# speedup vs baseline: 1.4168x; 1.4168x over previous
"""Trainium2 Bass kernel for the Griffin-style gated linear recurrence.

Model (matching the jax reference, including its chunked-scan numerics):
    a = sigmoid(x @ Wa.T + decay_bias)
    i = sigmoid(x @ Wi.T)
    v = x @ Wv.T
    w = sqrt(max(1 - a*a, 1e-8)) * i * v
    chunked scan (chunk=64), algebraically equal to
    h[t] = a[t]*h[t-1] + g[t]*w[t],  g[t] = min(1, cd[t]*1e10),
    cd = within-chunk running product of a (reset every 64 steps).

Sharding: 4 batches x 2 channel-halves = 8 cores, no communication.

Per-core layout: channels on partitions, time on the free axis, blocks of
SB=512 timesteps.  The 576 projection columns (3 gates x 192 channels) are
packed into 5 stationary tiles per k-tile instead of 6:
    z0 = a[0:128]   z1 = i[0:128]   z2 = v[0:128]
    z3 = [a[128:192] ; i[128:192]]  (64+64 partitions, one sigmoid with a
                                     [bias_hi ; 0] per-partition bias)
    z4 = v[128:192]                  (64 partitions)
which cuts tensor-engine time by 1/6 (matmul cost is N-cycles per
instruction regardless of M).  Inputs stream in bf16 (PSUM accumulates
fp32); the decay path (a, a^2, cd, h) stays fp32 end-to-end because
sqrt(1-a^2) for slow channels is catastrophically cancellation-sensitive.
The iv path runs bf16 where both operands are 2-byte (2x DVE mode).

z3 holds a_hi/i_hi on different partitions, so one cross-partition
SBUF->SBUF DMA per block realigns i_hi with v_hi (engines are lane-locked;
only DMA can move data across partitions).

Engine balance per block (PE is the bottleneck at ~8.5us/block), using
HW-measured per-op costs from traces of earlier revisions:
    PE  : 40 bf16 matmuls (5 tiles x 8 k-steps, N=512).
    Act : 3 sigmoids + 2 sqrt + ~2 act-table loads (sigmoid and sqrt never
          share a hardware table; the Tile scheduler is table-blind so the
          swap per block is unavoidable).
    DVE : u muls (PSUM reads), w muls (bf16 2x), g clamps (tensor_scalar
          measured 415ns here vs 7.5us (!) on Pool's Q7 software path),
          gw muls, one masked cd scan per group, 2 h scans (scan is
          DVE-only; the ISA rejects it on Pool).  Scan operands stay fp32:
          a bf16 data1 was measured to double scan time.
    Pool: a^2 squares and the cd mask muls (tensor_tensor ~1.15us there).

cd is computed as ONE scan per group instead of 8 64-column chunk scans:
    state = a_m[t]*state + a_s[t],  a_m = a*mask, a_s = a*maskc,
where mask zeroes the chunk-start columns (so the scan self-resets to
a[t] every 64 steps; ~270ns fixed cost per scan instruction made the
8-per-group version a DVE bottleneck).  The masks arrive as host inputs.
"""

import sys

if "/opt/trn_rl_repo" not in sys.path:
    sys.path.insert(0, "/opt/trn_rl_repo")

from contextlib import ExitStack

import numpy as np
import ml_dtypes

from concourse import bacc, bass, mybir, tile
from concourse.bass_utils import run_bass_kernel_spmd

B, S = 4, 4096
DM, DR = 1024, 384
DC = DR // 2          # channels per core
CH = 64               # scan chunk size
SB = 512              # sequence block per pipeline step
NB = S // SB
KT = DM // 128        # contraction tiles

F32 = mybir.dt.float32
BF16 = mybir.dt.bfloat16
AFT = mybir.ActivationFunctionType
OP = mybir.AluOpType

# column ranges of the 5 packed stationary tiles
TCOLS = ((0, 128), (128, 256), (256, 384), (384, 512), (512, 576))

_CACHED_NC = None


def _build_nc():
    nc = bacc.Bacc(trn_type="TRN2")

    xT = nc.dram_tensor("xt", [DM, S], BF16, kind="ExternalInput")
    wc = nc.dram_tensor("wcat", [DM, 576], BF16, kind="ExternalInput")
    bias0 = nc.dram_tensor("bias0", [128, 1], F32, kind="ExternalInput")
    bias3 = nc.dram_tensor("bias3", [128, 1], F32, kind="ExternalInput")
    maskm = nc.dram_tensor("maskm", [128, SB], F32, kind="ExternalInput")
    maskc = nc.dram_tensor("maskc", [128, SB], F32, kind="ExternalInput")
    out = nc.dram_tensor("out", [DC, S], F32, kind="ExternalOutput")

    with tile.TileContext(nc) as tc, ExitStack() as ctx:
        wp = ctx.enter_context(tc.tile_pool(name="wp", bufs=1))
        cp = ctx.enter_context(tc.tile_pool(name="cp", bufs=1))
        xp = ctx.enter_context(tc.tile_pool(name="xp", bufs=3))
        pp = ctx.enter_context(tc.tile_pool(name="pp", bufs=1, space="PSUM"))
        pv = ctx.enter_context(tc.tile_pool(name="pv", bufs=2, space="PSUM"))
        sp = ctx.enter_context(tc.tile_pool(name="sp", bufs=2))
        hp = ctx.enter_context(tc.tile_pool(name="hp", bufs=2))

        # --- constants -------------------------------------------------
        w_sb = wp.tile([128, KT, 576], BF16, tag="wc")
        nc.sync.dma_start(w_sb[:], wc.rearrange("(k p) c -> p k c", p=128))

        b0 = cp.tile([128, 1], F32, tag="b0")
        nc.sync.dma_start(b0[:], bias0[:, :])
        b3 = cp.tile([128, 1], F32, tag="b3")
        nc.sync.dma_start(b3[:], bias3[:, :])

        mm = cp.tile([128, SB], F32, tag="mm")
        nc.sync.dma_start(mm[:], maskm[:, :])
        mc = cp.tile([128, SB], F32, tag="mc")
        nc.sync.dma_start(mc[:], maskc[:, :])

        prev_h = None
        for ib in range(NB):
            s0 = ib * SB

            x_sb = xp.tile([128, KT, SB], BF16, tag="x")
            nc.sync.dma_start(
                x_sb[:],
                xT.rearrange("(k p) s -> p k s", p=128)[:, :, s0:s0 + SB])

            z = []
            for t, (c0, c1) in enumerate(TCOLS):
                # v tiles (z2, z4) drain mid-chain on DVE; double-buffer
                # them so next block's matmuls never wait (3 + 2*2 = 7 of
                # the 8 PSUM banks).
                pool = pv if t in (2, 4) else pp
                zt = pool.tile([c1 - c0, SB], F32, tag=f"z{t}")
                for k in range(KT):
                    nc.tensor.matmul(
                        zt[:],
                        w_sb[:, k, c0:c1],
                        x_sb[:, k, :],
                        start=(k == 0),
                        stop=(k == KT - 1),
                    )
                z.append(zt)

            # sigmoids (drain z0, z1, z3)
            a_lo = sp.tile([128, SB], F32, tag="a_lo")
            nc.scalar.activation(a_lo[:], z[0][:], AFT.Sigmoid, bias=b0[:])
            i_lo = sp.tile([128, SB], F32, tag="i_lo")
            nc.scalar.activation(i_lo[:], z[1][:], AFT.Sigmoid)
            s3 = sp.tile([128, SB], F32, tag="s3")     # [a_hi ; i_hi]
            nc.scalar.activation(s3[:], z[3][:], AFT.Sigmoid, bias=b3[:])
            a_hi = s3[0:64, :]

            # realign i_hi (partitions 64:128) with v_hi (partitions 0:64)
            ic = sp.tile([64, SB], F32, tag="ic")
            nc.sync.dma_start(ic[:], s3[64:128, :])

            # squares on Pool
            m_lo = sp.tile([128, SB], F32, tag="m_lo")
            nc.gpsimd.tensor_mul(m_lo[:], a_lo[:], a_lo[:])
            m_hi = sp.tile([64, SB], F32, tag="m_hi")
            nc.gpsimd.tensor_mul(m_hi[:], a_hi, a_hi)

            # r = sqrt(1 - a^2); 1 - a*a stays far above the reference's
            # 1e-8 floor for every reachable a, so the max() is a no-op.
            r_lo = sp.tile([128, SB], BF16, tag="r_lo")
            nc.scalar.activation(r_lo[:], m_lo[:], AFT.Sqrt, bias=1.0,
                                 scale=-1.0)
            r_hi = sp.tile([64, SB], BF16, tag="r_hi")
            nc.scalar.activation(r_hi[:], m_hi[:], AFT.Sqrt, bias=1.0,
                                 scale=-1.0)

            # u = i * v straight out of PSUM (drains z2, z4)
            u_lo = sp.tile([128, SB], BF16, tag="u_lo")
            nc.vector.tensor_mul(u_lo[:], i_lo[:], z[2][:])
            u_hi = sp.tile([64, SB], BF16, tag="u_hi")
            nc.vector.tensor_mul(u_hi[:], ic[:], z[4][:])

            # within-chunk running product of a via one masked scan per
            # group: state = (a*mask)[t]*state + (a*maskc)[t]; the zeroed
            # chunk-start column resets the product to a[t].
            am_lo = sp.tile([128, SB], F32, tag="am_lo")
            nc.gpsimd.tensor_mul(am_lo[:], a_lo[:], mm[:])
            as_lo = sp.tile([128, SB], F32, tag="as_lo")
            nc.gpsimd.tensor_mul(as_lo[:], a_lo[:], mc[:])
            am_hi = sp.tile([64, SB], F32, tag="am_hi")
            nc.gpsimd.tensor_mul(am_hi[:], a_hi, mm[0:64, :])
            as_hi = sp.tile([64, SB], F32, tag="as_hi")
            nc.gpsimd.tensor_mul(as_hi[:], a_hi, mc[0:64, :])

            cd_lo = sp.tile([128, SB], F32, tag="cd_lo")
            nc.vector.tensor_tensor_scan(
                cd_lo[:], am_lo[:], as_lo[:], 1.0, op0=OP.mult, op1=OP.add)
            cd_hi = sp.tile([64, SB], F32, tag="cd_hi")
            nc.vector.tensor_tensor_scan(
                cd_hi[:], am_hi[:], as_hi[:], 1.0, op0=OP.mult, op1=OP.add)

            w_lo = sp.tile([128, SB], BF16, tag="w_lo")
            nc.vector.tensor_mul(w_lo[:], r_lo[:], u_lo[:])
            w_hi = sp.tile([64, SB], BF16, tag="w_hi")
            nc.vector.tensor_mul(w_hi[:], r_hi[:], u_hi[:])

            # g = min(cd * 1e10, 1) == cd / max(cd, 1e-10)
            g_lo = sp.tile([128, SB], F32, tag="g_lo")
            nc.vector.tensor_scalar(
                g_lo[:], cd_lo[:], 1e10, 1.0, op0=OP.mult, op1=OP.min)
            g_hi = sp.tile([64, SB], F32, tag="g_hi")
            nc.vector.tensor_scalar(
                g_hi[:], cd_hi[:], 1e10, 1.0, op0=OP.mult, op1=OP.min)

            # gw stays fp32: a bf16 data1 halves h-scan throughput
            gw_lo = sp.tile([128, SB], F32, tag="gw_lo")
            nc.vector.tensor_mul(gw_lo[:], g_lo[:], w_lo[:])
            gw_hi = sp.tile([64, SB], F32, tag="gw_hi")
            nc.vector.tensor_mul(gw_hi[:], g_hi[:], w_hi[:])

            h_lo = hp.tile([128, SB], F32, tag="h_lo")
            init_lo = 0.0 if prev_h is None else prev_h[0][:, SB - 1:SB]
            nc.vector.tensor_tensor_scan(
                h_lo[:], a_lo[:], gw_lo[:], init_lo, op0=OP.mult, op1=OP.add)
            h_hi = hp.tile([64, SB], F32, tag="h_hi")
            init_hi = 0.0 if prev_h is None else prev_h[1][:, SB - 1:SB]
            nc.vector.tensor_tensor_scan(
                h_hi[:], s3[0:64, :], gw_hi[:], init_hi,
                op0=OP.mult, op1=OP.add)

            nc.sync.dma_start(out[0:128, s0:s0 + SB], h_lo[:])
            nc.sync.dma_start(out[128:DC, s0:s0 + SB], h_hi[:])
            prev_h = (h_lo, h_hi)

    nc.finalize()
    return nc


def _make_in_maps(x, Wa, Wi, Wv, decay_bias):
    x = np.asarray(x, dtype=np.float32)
    Wa = np.asarray(Wa, dtype=np.float32)
    Wi = np.asarray(Wi, dtype=np.float32)
    Wv = np.asarray(Wv, dtype=np.float32)
    decay_bias = np.asarray(decay_bias, dtype=np.float32)

    mask_m = np.ones((128, SB), dtype=np.float32)
    mask_m[:, 0::CH] = 0.0
    mask_c = np.zeros((128, SB), dtype=np.float32)
    mask_c[:, 0::CH] = 1.0

    in_maps = []
    for b in range(B):
        xTb = np.ascontiguousarray(x[b].T).astype(ml_dtypes.bfloat16)
        for j in range(2):
            c0 = j * DC
            wcat = np.concatenate(
                [
                    Wa[c0:c0 + 128].T,
                    Wi[c0:c0 + 128].T,
                    Wv[c0:c0 + 128].T,
                    Wa[c0 + 128:c0 + DC].T,
                    Wi[c0 + 128:c0 + DC].T,
                    Wv[c0 + 128:c0 + DC].T,
                ],
                axis=1,
            ).astype(ml_dtypes.bfloat16)
            b0 = np.ascontiguousarray(decay_bias[c0:c0 + 128, None])
            b3 = np.zeros((128, 1), dtype=np.float32)
            b3[0:64, 0] = decay_bias[c0 + 128:c0 + DC]
            in_maps.append({
                "xt": xTb,
                "wcat": np.ascontiguousarray(wcat),
                "bias0": b0,
                "bias3": b3,
                "maskm": mask_m,
                "maskc": mask_c,
            })
    return in_maps


def kernel(x, Wa, Wi, Wv, decay_bias):
    global _CACHED_NC
    if _CACHED_NC is None:
        _CACHED_NC = _build_nc()
    nc = _CACHED_NC

    in_maps = _make_in_maps(x, Wa, Wi, Wv, decay_bias)
    res = run_bass_kernel_spmd(nc, in_maps, core_ids=list(range(8)))

    out = np.empty((B, S, DR), dtype=np.float32)
    for b in range(B):
        for j in range(2):
            core = 2 * b + j
            out[b, :, j * DC:(j + 1) * DC] = res.results[core]["out"].T
    return out


# revision 10
# speedup vs baseline: 1.5023x; 1.0604x over previous
"""Trainium2 Bass kernel for the Griffin-style gated linear recurrence.

Model (matching the jax reference, including its chunked-scan numerics):
    a = sigmoid(x @ Wa.T + decay_bias)
    i = sigmoid(x @ Wi.T)
    v = x @ Wv.T
    w = sqrt(max(1 - a*a, 1e-8)) * i * v
    chunked scan (chunk=64), algebraically equal to
    h[t] = a[t]*h[t-1] + g[t]*w[t],  g[t] = min(1, cd[t]*1e10),
    cd = within-chunk running product of a (reset every 64 steps).

Sharding: 4 batches x 2 channel-halves = 8 cores, no communication.

Per-core layout: channels on partitions, time on the free axis, blocks of
SB=512 timesteps.  The 576 projection columns (3 gates x 192 channels) are
packed into 5 stationary tiles per k-tile instead of 6:
    z0 = a[0:128]   z1 = i[0:128]   z2 = v[0:128]
    z3 = [a[128:192] ; i[128:192]]  (64+64 partitions, one sigmoid with a
                                     [bias_hi ; 0] per-partition bias)
    z4 = v[128:192]                  (64 partitions)
which cuts tensor-engine time by 1/6 (matmul cost is N-cycles per
instruction regardless of M).  Inputs stream in bf16 (PSUM accumulates
fp32); the decay path (a, a^2, cd, h) stays fp32 end-to-end because
sqrt(1-a^2) for slow channels is catastrophically cancellation-sensitive.
The iv path runs bf16 where both operands are 2-byte (2x DVE mode).

z3 holds a_hi/i_hi on different partitions, so one cross-partition
SBUF->SBUF DMA per block realigns i_hi with v_hi (engines are lane-locked;
only DMA can move data across partitions).

Engine balance per block (PE is the bottleneck at ~8.5us/block), using
HW-measured per-op costs from traces of earlier revisions:
    PE  : 40 bf16 matmuls (5 tiles x 8 k-steps, N=512).
    Act : 3 sigmoids + 2 sqrt + ~2 act-table loads (sigmoid and sqrt never
          share a hardware table; the Tile scheduler is table-blind so the
          swap per block is unavoidable).
    DVE : u muls (PSUM reads), w muls (bf16 2x), g clamps (tensor_scalar
          measured 415ns here vs 7.5us (!) on Pool's Q7 software path),
          gw muls, one masked cd scan per group, 2 h scans (scan is
          DVE-only; the ISA rejects it on Pool).  Scan operands stay fp32:
          a bf16 data1 was measured to double scan time.
    Pool: a^2 squares and the cd mask muls (tensor_tensor ~1.15us there).

cd is computed as ONE scan per group instead of 8 64-column chunk scans:
    state = a_m[t]*state + a_s[t]
where a_m is a with the chunk-start columns zeroed (so the scan
self-resets to a[t] every 64 steps; ~270ns fixed cost per scan
instruction made the 8-per-group version a DVE bottleneck).  a_m is a
Pool copy + an 8-column strided memset; a_s holds a's chunk-start
columns in an otherwise-zero tile (zeroed once at startup, the same 8
columns rewritten every block) -- no full-tile mask multiplies and no
mask inputs.

Startup: weights stream as 8 per-k tiles and x via the Act engine's DMA
queue, so the first matmul starts at ~3.5us instead of waiting ~19us
behind one serialized constant-upload queue.
"""

import sys

if "/opt/trn_rl_repo" not in sys.path:
    sys.path.insert(0, "/opt/trn_rl_repo")

from contextlib import ExitStack

import numpy as np
import ml_dtypes

from concourse import bacc, bass, mybir, tile
from concourse.bass_utils import run_bass_kernel_spmd

B, S = 4, 4096
DM, DR = 1024, 384
DC = DR // 2          # channels per core
CH = 64               # scan chunk size
SB = 512              # sequence block per pipeline step
NB = S // SB
KT = DM // 128        # contraction tiles

F32 = mybir.dt.float32
BF16 = mybir.dt.bfloat16
AFT = mybir.ActivationFunctionType
OP = mybir.AluOpType

# column ranges of the 5 packed stationary tiles
TCOLS = ((0, 128), (128, 256), (256, 384), (384, 512), (512, 576))

_CACHED_NC = None


def _build_nc():
    nc = bacc.Bacc(trn_type="TRN2")

    xT = nc.dram_tensor("xt", [DM, S], BF16, kind="ExternalInput")
    wc = nc.dram_tensor("wcat", [DM, 576], BF16, kind="ExternalInput")
    bias0 = nc.dram_tensor("bias0", [128, 1], F32, kind="ExternalInput")
    bias3 = nc.dram_tensor("bias3", [128, 1], F32, kind="ExternalInput")
    out = nc.dram_tensor("out", [DC, S], F32, kind="ExternalOutput")

    with tile.TileContext(nc) as tc, ExitStack() as ctx:
        wp = ctx.enter_context(tc.tile_pool(name="wp", bufs=1))
        cp = ctx.enter_context(tc.tile_pool(name="cp", bufs=1))
        xp = ctx.enter_context(tc.tile_pool(name="xp", bufs=3))
        pp = ctx.enter_context(tc.tile_pool(name="pp", bufs=1, space="PSUM"))
        pv = ctx.enter_context(tc.tile_pool(name="pv", bufs=2, space="PSUM"))
        sp = ctx.enter_context(tc.tile_pool(name="sp", bufs=2))
        ap = ctx.enter_context(tc.tile_pool(name="ap", bufs=2))
        hp = ctx.enter_context(tc.tile_pool(name="hp", bufs=2))

        # --- constants -------------------------------------------------
        # per-k weight tiles so the first matmuls gate on 144KB, not 1.15MB
        w_k = []
        for k in range(KT):
            wk = wp.tile([128, 576], BF16, tag=f"wk{k}")
            nc.sync.dma_start(wk[:], wc[k * 128:(k + 1) * 128, :])
            w_k.append(wk)

        b0 = cp.tile([128, 1], F32, tag="b0")
        nc.scalar.dma_start(b0[:], bias0[:, :])
        b3 = cp.tile([128, 1], F32, tag="b3")
        nc.scalar.dma_start(b3[:], bias3[:, :])

        prev_h = None
        for ib in range(NB):
            s0 = ib * SB

            x_sb = xp.tile([128, KT, SB], BF16, tag="x")
            nc.scalar.dma_start(
                x_sb[:],
                xT.rearrange("(k p) s -> p k s", p=128)[:, :, s0:s0 + SB])

            z = []
            for t, (c0, c1) in enumerate(TCOLS):
                # v tiles (z2, z4) drain mid-chain on DVE; double-buffer
                # them so next block's matmuls never wait (3 + 2*2 = 7 of
                # the 8 PSUM banks).
                pool = pv if t in (2, 4) else pp
                zt = pool.tile([c1 - c0, SB], F32, tag=f"z{t}")
                for k in range(KT):
                    nc.tensor.matmul(
                        zt[:],
                        w_k[k][:, c0:c1],
                        x_sb[:, k, :],
                        start=(k == 0),
                        stop=(k == KT - 1),
                    )
                z.append(zt)

            # sigmoids (drain z0, z1, z3)
            a_lo = sp.tile([128, SB], F32, tag="a_lo")
            nc.scalar.activation(a_lo[:], z[0][:], AFT.Sigmoid, bias=b0[:])
            i_lo = sp.tile([128, SB], BF16, tag="i_lo")
            nc.scalar.activation(i_lo[:], z[1][:], AFT.Sigmoid)
            s3 = sp.tile([128, SB], F32, tag="s3")     # [a_hi ; i_hi]
            nc.scalar.activation(s3[:], z[3][:], AFT.Sigmoid, bias=b3[:])
            a_hi = s3[0:64, :]

            # realign i_hi (partitions 64:128) with v_hi (partitions 0:64)
            ic = sp.tile([64, SB], F32, tag="ic")
            nc.sync.dma_start(ic[:], s3[64:128, :])

            # squares on Pool
            m_lo = sp.tile([128, SB], F32, tag="m_lo")
            nc.gpsimd.tensor_mul(m_lo[:], a_lo[:], a_lo[:])
            m_hi = sp.tile([64, SB], F32, tag="m_hi")
            nc.gpsimd.tensor_mul(m_hi[:], a_hi, a_hi)

            # r = sqrt(1 - a^2); 1 - a*a stays far above the reference's
            # 1e-8 floor for every reachable a, so the max() is a no-op.
            r_lo = sp.tile([128, SB], BF16, tag="r_lo")
            nc.scalar.activation(r_lo[:], m_lo[:], AFT.Sqrt, bias=1.0,
                                 scale=-1.0)
            r_hi = sp.tile([64, SB], BF16, tag="r_hi")
            nc.scalar.activation(r_hi[:], m_hi[:], AFT.Sqrt, bias=1.0,
                                 scale=-1.0)

            # u = i * v straight out of PSUM (drains z2, z4)
            u_lo = sp.tile([128, SB], BF16, tag="u_lo")
            nc.vector.tensor_mul(u_lo[:], i_lo[:], z[2][:])
            u_hi = sp.tile([64, SB], BF16, tag="u_hi")
            nc.vector.tensor_mul(u_hi[:], ic[:], z[4][:])

            # within-chunk running product of a via one masked scan per
            # group: state = a_m[t]*state + a_s[t]; the zeroed chunk-start
            # columns of a_m reset the product to a[t] (from a_s).
            am_lo = sp.tile([128, SB], F32, tag="am_lo")
            nc.gpsimd.tensor_copy(am_lo[:], a_lo[:])
            nc.gpsimd.memset(am_lo[:, 0::CH], 0.0)
            am_hi = sp.tile([64, SB], F32, tag="am_hi")
            nc.gpsimd.tensor_copy(am_hi[:], a_hi)
            nc.gpsimd.memset(am_hi[:, 0::CH], 0.0)
            # a_s slots are zeroed once (first two blocks); afterwards only
            # the same 8 chunk-start columns are rewritten each block.
            as_lo = ap.tile([128, SB], F32, tag="as_lo")
            as_hi = ap.tile([64, SB], F32, tag="as_hi")
            if ib < 2:
                nc.gpsimd.memset(as_lo[:], 0.0)
                nc.gpsimd.memset(as_hi[:], 0.0)
            nc.gpsimd.tensor_copy(as_lo[:, 0::CH], a_lo[:, 0::CH])
            nc.gpsimd.tensor_copy(as_hi[:, 0::CH], a_hi[:, 0::CH])

            cd_lo = sp.tile([128, SB], F32, tag="cd_lo")
            nc.vector.tensor_tensor_scan(
                cd_lo[:], am_lo[:], as_lo[:], 1.0, op0=OP.mult, op1=OP.add)
            cd_hi = sp.tile([64, SB], F32, tag="cd_hi")
            nc.vector.tensor_tensor_scan(
                cd_hi[:], am_hi[:], as_hi[:], 1.0, op0=OP.mult, op1=OP.add)

            w_lo = sp.tile([128, SB], BF16, tag="w_lo")
            nc.vector.tensor_mul(w_lo[:], r_lo[:], u_lo[:])
            w_hi = sp.tile([64, SB], BF16, tag="w_hi")
            nc.vector.tensor_mul(w_hi[:], r_hi[:], u_hi[:])

            # g = min(cd * 1e10, 1) == cd / max(cd, 1e-10)
            g_lo = sp.tile([128, SB], F32, tag="g_lo")
            nc.vector.tensor_scalar(
                g_lo[:], cd_lo[:], 1e10, 1.0, op0=OP.mult, op1=OP.min)
            g_hi = sp.tile([64, SB], F32, tag="g_hi")
            nc.vector.tensor_scalar(
                g_hi[:], cd_hi[:], 1e10, 1.0, op0=OP.mult, op1=OP.min)

            # gw stays fp32: a bf16 data1 halves h-scan throughput
            gw_lo = sp.tile([128, SB], F32, tag="gw_lo")
            nc.vector.tensor_mul(gw_lo[:], g_lo[:], w_lo[:])
            gw_hi = sp.tile([64, SB], F32, tag="gw_hi")
            nc.vector.tensor_mul(gw_hi[:], g_hi[:], w_hi[:])

            h_lo = hp.tile([128, SB], F32, tag="h_lo")
            init_lo = 0.0 if prev_h is None else prev_h[0][:, SB - 1:SB]
            nc.vector.tensor_tensor_scan(
                h_lo[:], a_lo[:], gw_lo[:], init_lo, op0=OP.mult, op1=OP.add)
            h_hi = hp.tile([64, SB], F32, tag="h_hi")
            init_hi = 0.0 if prev_h is None else prev_h[1][:, SB - 1:SB]
            nc.vector.tensor_tensor_scan(
                h_hi[:], s3[0:64, :], gw_hi[:], init_hi,
                op0=OP.mult, op1=OP.add)

            nc.sync.dma_start(out[0:128, s0:s0 + SB], h_lo[:])
            nc.sync.dma_start(out[128:DC, s0:s0 + SB], h_hi[:])
            prev_h = (h_lo, h_hi)

    nc.finalize()
    return nc


def _make_in_maps(x, Wa, Wi, Wv, decay_bias):
    x = np.asarray(x, dtype=np.float32)
    Wa = np.asarray(Wa, dtype=np.float32)
    Wi = np.asarray(Wi, dtype=np.float32)
    Wv = np.asarray(Wv, dtype=np.float32)
    decay_bias = np.asarray(decay_bias, dtype=np.float32)

    in_maps = []
    for b in range(B):
        xTb = np.ascontiguousarray(x[b].T).astype(ml_dtypes.bfloat16)
        for j in range(2):
            c0 = j * DC
            wcat = np.concatenate(
                [
                    Wa[c0:c0 + 128].T,
                    Wi[c0:c0 + 128].T,
                    Wv[c0:c0 + 128].T,
                    Wa[c0 + 128:c0 + DC].T,
                    Wi[c0 + 128:c0 + DC].T,
                    Wv[c0 + 128:c0 + DC].T,
                ],
                axis=1,
            ).astype(ml_dtypes.bfloat16)
            b0 = np.ascontiguousarray(decay_bias[c0:c0 + 128, None])
            b3 = np.zeros((128, 1), dtype=np.float32)
            b3[0:64, 0] = decay_bias[c0 + 128:c0 + DC]
            in_maps.append({
                "xt": xTb,
                "wcat": np.ascontiguousarray(wcat),
                "bias0": b0,
                "bias3": b3,
            })
    return in_maps


def kernel(x, Wa, Wi, Wv, decay_bias):
    global _CACHED_NC
    if _CACHED_NC is None:
        _CACHED_NC = _build_nc()
    nc = _CACHED_NC

    in_maps = _make_in_maps(x, Wa, Wi, Wv, decay_bias)
    res = run_bass_kernel_spmd(nc, in_maps, core_ids=list(range(8)))

    out = np.empty((B, S, DR), dtype=np.float32)
    for b in range(B):
        for j in range(2):
            core = 2 * b + j
            out[b, :, j * DC:(j + 1) * DC] = res.results[core]["out"].T
    return out
